# revision 13
# baseline (speedup 1.0000x reference)
"""Sliding-window causal attention (B=2, H=16, T=2048, D=64, WINDOW=512) on
8 TRN2 NeuronCores.

Sharding: the 32 (b, h) pairs are split 4-per-core (embarrassingly parallel).
Each core runs the same Bass/Tile program over its 4 heads (2 pairs).

v2 redesign (vs 88us baseline):
  - Triangle masking moved from DVE tensor_muls to PE "penalty matmuls":
    identity-weight x [L|U] (-240) pattern accumulated into the score PSUM.
    exp(score-240*SCALE) ~ 0, so masked cells vanish in both PV numerator
    and the ones-column denominator.  Saves ~17us of DVE time.
  - The two heads of a pair share one score PSUM tile [128,1536] (3 banks,
    2 bufs): head A in cols 0:512 + 1024:1152, head B 512:1024 + 1152:1280.
    exp becomes ONE wide instruction per (pair, kb) instead of two,
    amortizing the ~352cy ACT instruction overhead.
  - exp is split between ACT (true exp, leading columns) and DVE
    (Schraudolph bit-trick: bits = round(score*SCALE*184.665 + 16250.5)
    as int16 == bf16 bits of exp; ~3% per-element, ~4e-3 end-to-end).
  - Normalization merged: one reciprocal [128,4] + one 4D tensor_mul per
    2 query blocks covering both heads.
  - Staging casts moved off DVE (ACT during bootstrap, GPSIMD in feed).
"""

import os
import sys
from contextlib import ExitStack

import numpy as np

sys.path.insert(0, "/opt/trn_rl_repo")

import concourse.bacc as bacc
import concourse.tile as tile
from concourse import mybir
from concourse.bass_utils import run_bass_kernel_spmd

F32 = mybir.dt.float32
BF16 = mybir.dt.bfloat16
I16 = mybir.dt.int16
EXP = mybir.ActivationFunctionType.Exp
MULT = mybir.AluOpType.mult
ADD = mybir.AluOpType.add

B, H, T, D = 2, 16, 2048, 64
WINDOW = 512
SCALE = D ** -0.5
N_CORES = 8
HEADS_PER_CORE = (B * H) // N_CORES  # 4
TB = T // 128  # 16 query/key blocks
TH = 1024  # half-sequence tile width for qd/kd

# Schraudolph exp: bf16 bits of exp(t) ~ round(184.665*t + 16250.5)
A_MULT = SCALE * 128 * 1.4426950408889634
A_ADD = 16250.5
PEN = -240.0  # additive pre-scale penalty: exp(PEN*SCALE) = e^-30 ~ 0

# columns of each merged exp tile handled by DVE (Schraudolph); rest on ACT
DVE_COLS = int(os.environ.get("KOPT_DVE_COLS", "448"))
FEED_CAST = os.environ.get("KOPT_FEED_CAST", "gps")
USE_PEN = os.environ.get("KOPT_PEN", "1") == "1"
WIDE_EXP = os.environ.get("KOPT_WIDE", "1") == "1"
NORM4D = os.environ.get("KOPT_NORM4D", "1") == "1"


def ecol(hl, j):
    """Column offset of the 128-wide E chunk for head-local hl (0/1) and
    j = qb - kb (0..4) in the merged [128, 1280] tile."""
    if j == 0:
        return 512 * hl          # diag
    if j == 4:
        return 512 * hl + 128    # window boundary
    if j < 3:
        return 512 * hl + 256 + 128 * (j - 1)  # mid j=1,2
    return 1024 + 128 * hl       # mid j=3


def build_nc(t=T, heads_per_core=HEADS_PER_CORE):
    nb = t // 128

    nc = bacc.Bacc("TRN2", target_bir_lowering=False)
    q_ext = nc.declare_dram_parameter("q", [heads_per_core, t, D], F32, isOutput=False)
    k_ext = nc.declare_dram_parameter("k", [heads_per_core, t, D], F32, isOutput=False)
    v_ext = nc.declare_dram_parameter("v", [heads_per_core, t, D], F32, isOutput=False)
    id_ext = nc.declare_dram_parameter("ident", [128, 128], F32, isOutput=False)
    pen_ext = nc.declare_dram_parameter("pen", [128, 256], F32, isOutput=False)
    o_ext = nc.declare_dram_parameter("out", [heads_per_core, t, D], F32, isOutput=True)

    assert heads_per_core % 2 == 0
    n_pairs = heads_per_core // 2

    with tile.TileContext(nc) as tc, ExitStack() as ctx:
        const = ctx.enter_context(tc.tile_pool(name="const", bufs=1))
        stage = ctx.enter_context(tc.tile_pool(name="stage", bufs=6))
        vstage = ctx.enter_context(tc.tile_pool(name="vstage", bufs=2))
        qkd = ctx.enter_context(tc.tile_pool(name="qkd", bufs=2))
        vps = ctx.enter_context(tc.tile_pool(name="vps", bufs=4))
        ets = ctx.enter_context(tc.tile_pool(name="ets", bufs=7))
        outs = ctx.enter_context(tc.tile_pool(name="outs", bufs=2))
        rcp = ctx.enter_context(tc.tile_pool(name="rcp", bufs=4))
        # PSUM banks: 1 (trp) + 2*3 (sp) + 1 (shared ob/warmup) = 8
        tr_ps = ctx.enter_context(tc.tile_pool(name="tr_ps", bufs=1, space="PSUM"))
        s_ps = ctx.enter_context(tc.tile_pool(name="s_ps", bufs=2, space="PSUM"))
        ob_ps = ctx.enter_context(tc.tile_pool(name="ob_ps", bufs=1, space="PSUM"))

        # HAM warmup: burn a dense burst of dummy matmuls on a scratch PSUM
        # region while the first DMAs are in flight so the PE clock gate
        # opens (1.2 -> 2.4 GHz) before the real pipeline starts.
        dm_src = const.tile([128, 128], BF16, tag="dm_src")
        nc.vector.memset(dm_src[:], 0.0)
        dm_out = ob_ps.tile([128, 512], F32, tag="ob", name="ob_warm")

        def pe_dummy(n):
            for i in range(n):
                nc.tensor.matmul(
                    dm_out[:, 384:512], dm_src[:], dm_src[:], start=True, stop=True
                )

        # fp32 identity + bf16 copy (for Q/K transposes + penalty matmuls).
        ident_f = const.tile([128, 128], F32, tag="ident_f")
        nc.sync.dma_start(out=ident_f[:], in_=id_ext[:])
        ident_b = const.tile([128, 128], BF16, tag="ident_b")
        nc.vector.tensor_copy(ident_b[:], ident_f[:])

        # penalty pattern [L | U]: L = PEN where col<ch (kills q<k in diag),
        # U = PEN where col>=ch (kills out-of-window in win chunk).
        pen_f = const.tile([128, 256], F32, tag="pen_f")
        nc.scalar.dma_start(out=pen_f[:], in_=pen_ext[:])
        pen_b = const.tile([128, 256], BF16, tag="pen_b")
        nc.vector.tensor_copy(pen_b[:], pen_f[:])
        mask01 = const.tile([128, 256], BF16, tag="mask01")
        nc.vector.tensor_scalar(
            mask01[:], pen_f[:], -1.0 / 240.0, 1.0, MULT, ADD
        )

        # per-pair state
        qd_halves = {}
        kd_halves = {}
        vp = {}

        def alloc_pair(pair):
            qd_halves[pair] = [
                qkd.tile([128, TH], BF16, tag="qd0", name=f"qd0_{pair}"),
                qkd.tile([128, TH], BF16, tag="qd1", name=f"qd1_{pair}"),
            ]
            kd_halves[pair] = [
                qkd.tile([128, TH], BF16, tag="kd0", name=f"kd0_{pair}"),
                qkd.tile([128, TH], BF16, tag="kd1", name=f"kd1_{pair}"),
            ]

        def stage_dma(pair, ext, u, engs):
            # DMA one 512-row chunk of q or k (both heads) into a staging
            # tile; issue the two half-DMAs on separate engine queues.
            rows = slice(u * 512, (u + 1) * 512)
            st_f = stage.tile([128, 512], F32, tag="st_f")
            st3 = st_f[:].rearrange("p (b c) -> p b c", c=128)
            for eng, (hh, doff) in zip(
                engs, ((2 * pair, 0), (2 * pair + 1, 64))
            ):
                eng.dma_start(
                    out=st3[:, :, doff : doff + 64],
                    in_=ext[hh, rows, :].rearrange("(b p) d -> p b d", p=128),
                )
            return st_f

        def stage_compute(st_f, halves, u, cast="gps"):
            # cast -> 4 PE transposes -> drain into the d-major half
            st_b = stage.tile([128, 512], BF16, tag="st_b")
            if cast == "act":
                nc.scalar.activation(
                    st_b[:], st_f[:], mybir.ActivationFunctionType.Copy
                )
            elif cast == "gps":
                nc.gpsimd.tensor_copy(st_b[:], st_f[:])
            else:
                nc.vector.tensor_copy(st_b[:], st_f[:])
            trp = tr_ps.tile([128, 512], BF16, tag="trp")
            for i in range(4):
                nc.tensor.transpose(
                    trp[:, i * 128 : (i + 1) * 128],
                    st_b[:, i * 128 : (i + 1) * 128],
                    ident_b[:],
                )
            dst = halves[u // 2]
            dcol = (u % 2) * 512
            nc.vector.tensor_copy(dst[:, dcol : dcol + 512], trp[:, 0:512])

        def stage_unit(pair, ext, halves, u, engs, cast=FEED_CAST):
            stage_compute(stage_dma(pair, ext, u, engs), halves, u, cast=cast)

        def stage_v(h):
            vst = vstage.tile([128, 1024], F32, tag="vst")
            v3 = vst[:].rearrange("p (b d) -> p b d", d=64)
            nc.gpsimd.dma_start(
                out=v3, in_=v_ext[h].rearrange("(b p) d -> p b d", p=128)
            )
            vt = vps.tile([128, nb, 65], BF16, tag="vp", name=f"vp_{h}")
            nc.vector.tensor_copy(vt[:, :, 0:64], v3)
            nc.gpsimd.memset(vt[:, :, 64:65], 1.0)
            vp[h] = vt

        def stage_feed(pair, engs):
            alloc_pair(pair)
            units = []
            units.append(lambda: stage_unit(pair, q_ext, qd_halves[pair], 0, engs))
            units.append(lambda: stage_unit(pair, k_ext, kd_halves[pair], 0, engs))
            units.append(lambda: stage_v(2 * pair))
            units.append(lambda: stage_v(2 * pair + 1))
            units.append(lambda: stage_unit(pair, q_ext, qd_halves[pair], 1, engs))
            units.append(lambda: stage_unit(pair, k_ext, kd_halves[pair], 1, engs))
            for u in (2, 3):
                units.append(
                    lambda u=u: stage_unit(pair, q_ext, qd_halves[pair], u, engs)
                )
                units.append(
                    lambda u=u: stage_unit(pair, k_ext, kd_halves[pair], u, engs)
                )
            return units

        def attention(pair, feed):
            hA, hB = 2 * pair, 2 * pair + 1
            rows_of = {0: slice(0, 64), 1: slice(64, 128)}
            qdh, kdh = qd_halves[pair], kd_halves[pair]
            et = {}
            oo_t = [None]

            def emit_qk(kb):
                a = kb * 128
                has_win = a + 640 <= t
                mw12 = max(0, min(256, t - a - 128))
                mw3 = max(0, min(128, t - a - 384))
                sp = s_ps.tile([128, 1536], F32, tag="sp", name=f"sp_{pair}_{kb}")
                kd_half = kdh[a // TH]
                kcol = a % TH

                # per-head chunk lists: (ecol, qlo, n).  start=True clears the
                # has_written bits of the WHOLE bank, so only the first
                # matmul touching each bank may use it; later chunks
                # overwrite-where-clear / accumulate-where-set.
                def head_chunks(hl):
                    base = 512 * hl
                    ch = [(hl, base, a, 128)]  # diag (bank first; pen closes)
                    if has_win:
                        ch.append((hl, base + 128, a + 512, 128))
                    # mid j=1,2 (split at qd-half boundary)
                    q0 = a + 128
                    rem = mw12
                    c = base + 256
                    while rem > 0:
                        n = min(rem, TH - (q0 % TH))
                        ch.append((hl, c, q0, n))
                        q0 += n; c += n; rem -= n
                    return ch

                # zip A/B chunks for co-execution (they write different
                # banks); the two mid3 chunks share bank 2 and must NOT
                # co-execute (one PE write port per bank), so mid3A is
                # emitted first (adjacent to same-row diagA => serial) and
                # mid3B dead last.
                ordered = []
                if mw3 > 0:
                    ordered.append((0, 1024, a + 384, mw3))
                for ca, cb in zip(head_chunks(0), head_chunks(1)):
                    ordered.append(ca)
                    ordered.append(cb)
                if mw3 > 0:
                    ordered.append((1, 1152, a + 384, mw3))
                bank_started = set()
                for (hl, c, qlo, n) in ordered:
                    bank = c // 512
                    st_ = bank not in bank_started
                    bank_started.add(bank)
                    # diag/win cells are closed by the penalty matmul;
                    # mids close themselves.
                    sp_ = (c % 512) >= 256 or c >= 1024 or not USE_PEN
                    nc.tensor.matmul(
                        sp[:, c : c + n],
                        kd_half[r_ := rows_of[hl], kcol : kcol + 128],
                        qdh[qlo // TH][r_, qlo % TH : qlo % TH + n],
                        start=st_,
                        stop=sp_,
                    )
                # penalty accumulates onto diag+win (overwrites win cols with
                # the pattern when there is no win chunk -- never read then)
                if USE_PEN:
                    for hl in (0, 1):
                        nc.tensor.matmul(
                            sp[:, 512 * hl : 512 * hl + 256],
                            ident_b[:],
                            pen_b[:],
                            start=False,
                            stop=True,
                        )

                # written column runs of the merged tile
                bnd = 256 if (has_win or USE_PEN) else 128
                runs = []
                for base in (0, 512):
                    runs.append((base, base + bnd))
                    if mw12 > 0:
                        runs.append((base + 256, base + 256 + mw12))
                if mw3 > 0:
                    runs.append((1024, 1024 + mw3))
                    runs.append((1152, 1152 + mw3))
                merged = []
                for lo, hi in runs:
                    if merged and merged[-1][1] == lo:
                        merged[-1] = (merged[-1][0], hi)
                    else:
                        merged.append((lo, hi))

                e = ets.tile([128, 1280], BF16, tag="et", name=f"et_{pair}_{kb}")
                et[kb] = e
                for lo, hi in merged:
                    dcols = DVE_COLS if (hi - lo) >= 1024 else 0
                    split = hi - dcols
                    if not WIDE_EXP:
                        # split ACT part at 512-boundaries
                        c0 = lo
                        while c0 < split:
                            c1 = min(split, (c0 // 512 + 1) * 512)
                            nc.scalar.activation(
                                e[:, c0:c1], sp[:, c0:c1], EXP, scale=SCALE
                            )
                            c0 = c1
                    elif split > lo:
                        nc.scalar.activation(
                            e[:, lo:split], sp[:, lo:split], EXP, scale=SCALE
                        )
                    if dcols:
                        nc.vector.tensor_scalar(
                            e[:, split:hi].bitcast(I16),
                            sp[:, split:hi],
                            A_MULT,
                            A_ADD,
                            MULT,
                            ADD,
                        )
                if not USE_PEN:
                    for base in (0, 512):
                        nc.vector.tensor_mul(
                            e[:, base : base + bnd],
                            e[:, base : base + bnd],
                            mask01[:, 0:bnd],
                        )

            ob_t = {}

            def emit_pv(qb):
                g, j4 = qb // 4, qb % 4
                g2, j2 = qb // 2, qb % 2
                jj = (qb % 4) // 2
                for hl in (0, 1):
                    h = 2 * pair + hl
                    if j2 == 0 and hl == 0:
                        ob_t[0] = ob_ps.tile(
                            [128, 512], F32, tag="ob", name=f"ob_{pair}_{g2}"
                        )
                    hoff = 130 * hl
                    ob = ob_t[0][:, hoff : hoff + 130].rearrange(
                        "p (b c) -> p b c", c=65
                    )
                    kb0 = max(0, qb - 4)
                    for kb in range(kb0, qb + 1):
                        c = ecol(hl, qb - kb)
                        nc.tensor.matmul(
                            ob[:, j2, :],
                            et[kb][:, c : c + 128],
                            vp[h][:, kb, :],
                            start=(kb == kb0),
                            stop=(kb == qb),
                        )
                if qb >= 4:
                    del et[qb - 4]
                if j2 == 1:
                    if jj == 0:
                        oo_t[0] = outs.tile(
                            [128, 512], F32, tag="oo", name=f"oo_{pair}_{g}"
                        )
                    ob4 = ob_t[0][:, 0:260].rearrange(
                        "p (h b c) -> p h b c", h=2, c=65
                    )
                    oo4 = oo_t[0][:].rearrange(
                        "p (h b d) -> p h b d", h=2, d=64
                    )
                    if NORM4D:
                        rc = rcp.tile([128, 4], F32, tag="rc")
                        rc2 = rc[:].rearrange("p (h b) -> p h b", h=2)
                        nc.vector.reciprocal(rc2, ob4[:, :, :, 64])
                        nc.vector.tensor_mul(
                            oo4[:, :, 2 * jj : 2 * jj + 2, :],
                            ob4[:, :, :, 0:64],
                            rc[:]
                            .rearrange("p (h b c) -> p h b c", h=2, c=1)
                            .broadcast_to([128, 2, 2, 64]),
                        )
                    else:
                        for hl in (0, 1):
                            rc = rcp.tile([128, 2], F32, tag="rc")
                            nc.vector.reciprocal(rc[:], ob4[:, hl, :, 64])
                            nc.vector.tensor_mul(
                                oo4[:, hl, 2 * jj : 2 * jj + 2, :],
                                ob4[:, hl, :, 0:64],
                                rc[:]
                                .rearrange("p (b c) -> p b c", c=1)
                                .broadcast_to([128, 2, 64]),
                            )
                if j4 == 3:
                    oo4 = oo_t[0][:].rearrange(
                        "p (h b d) -> p h b d", h=2, d=64
                    )
                    for hl in (0, 1):
                        h = 2 * pair + hl
                        o_dst = o_ext[h, g * 512 : g * 512 + 512, :].rearrange(
                            "(b p) d -> p b d", p=128
                        )
                        if pair == n_pairs - 1 and g == 3:
                            nc.sync.dma_start(
                                out=o_dst[:, 0:2, :], in_=oo4[:, hl, 0:2, :]
                            )
                            nc.scalar.dma_start(
                                out=o_dst[:, 2:4, :], in_=oo4[:, hl, 2:4, :]
                            )
                        else:
                            nc.sync.dma_start(out=o_dst, in_=oo4[:, hl, :, :])

            for kb in range(nb + 1):
                if kb < nb:
                    emit_qk(kb)
                if kb >= 1:
                    emit_pv(kb - 1)
                for fn in feed.get(kb, ()):
                    fn()

        # bootstrap: QK(0) needs q rows 0:1024 (u0q,u1q) and k rows 0:512
        # (u0k) -- spread those three units across the sync+scalar HW-DGE
        # queues; V rides the gpsimd SWDGE queue.
        alloc_pair(0)
        stf_q0 = stage_dma(0, q_ext, 0, (nc.sync, nc.scalar))
        stf_k0 = stage_dma(0, k_ext, 0, (nc.sync, nc.scalar))
        stf_q1 = stage_dma(0, q_ext, 1, (nc.sync, nc.scalar))
        stage_v(0)
        stage_v(1)
        pe_dummy(30)
        stage_compute(stf_q0, qd_halves[0], 0, cast="dve")
        stage_compute(stf_k0, kd_halves[0], 0, cast="act")
        stage_compute(stf_q1, qd_halves[0], 1, cast="dve")
        dm_sink = const.tile([128, 1], F32, tag="dm_sink")
        nc.vector.tensor_copy(dm_sink[:], dm_out[:, 384:385])
        eng0 = (nc.sync, nc.gpsimd)
        feed0 = {
            0: [
                lambda: stage_unit(0, k_ext, kd_halves[0], 1, eng0),
                lambda: stage_unit(0, q_ext, qd_halves[0], 2, eng0),
            ],
            1: [lambda: stage_unit(0, k_ext, kd_halves[0], 2, eng0)],
            2: [lambda: stage_unit(0, q_ext, qd_halves[0], 3, eng0)],
            3: [lambda: stage_unit(0, k_ext, kd_halves[0], 3, eng0)],
        }
        # pair 1 staged during pair 0's attention, starting at kb=6
        units1 = stage_feed(1, eng0)
        feed1_in_0 = {6 + i: [units1[i]] for i in range(len(units1))}
        feed0.update(feed1_in_0)

        attention(0, feed0)
        attention(1, {})

    nc.compile()
    return nc


_NC_CACHE = {}
TRACE = False
TRACE_DIR = None
LAST_RESULT = None


def _get_nc():
    key = (T, HEADS_PER_CORE)
    if key not in _NC_CACHE:
        _NC_CACHE[key] = build_nc()
    return _NC_CACHE[key]


def _pen_pattern():
    ch = np.arange(128)[:, None]
    col = np.arange(128)[None, :]
    L = np.where(col < ch, PEN, 0.0).astype(np.float32)
    U = np.where(col >= ch, PEN, 0.0).astype(np.float32)
    return np.concatenate([L, U], axis=1)


def kernel(q, k, v):
    q = np.ascontiguousarray(np.asarray(q, dtype=np.float32))
    k = np.ascontiguousarray(np.asarray(k, dtype=np.float32))
    v = np.ascontiguousarray(np.asarray(v, dtype=np.float32))
    assert q.shape == (B, H, T, D)

    qf = q.reshape(B * H, T, D)
    kf = k.reshape(B * H, T, D)
    vf = v.reshape(B * H, T, D)
    ident = np.eye(128, dtype=np.float32)
    pen = _pen_pattern()

    in_maps = []
    for c in range(N_CORES):
        s = slice(c * HEADS_PER_CORE, (c + 1) * HEADS_PER_CORE)
        in_maps.append(
            {
                "q": np.ascontiguousarray(qf[s]),
                "k": np.ascontiguousarray(kf[s]),
                "v": np.ascontiguousarray(vf[s]),
                "ident": ident,
                "pen": pen,
            }
        )

    nc = _get_nc()
    global LAST_RESULT
    res = run_bass_kernel_spmd(
        nc, in_maps, list(range(N_CORES)), trace=TRACE, tmpdir=TRACE_DIR
    )
    LAST_RESULT = res
    out = np.concatenate([res.results[c]["out"] for c in range(N_CORES)], axis=0)
    return out.reshape(B, H, T, D).astype(np.float32)


# revision 16
# speedup vs baseline: 1.1641x; 1.1641x over previous
"""Sliding-window causal attention (B=2, H=16, T=2048, D=64, WINDOW=512) on
8 TRN2 NeuronCores.

Sharding: the 32 (b, h) pairs are split 4-per-core (embarrassingly parallel).
Each core runs the same Bass/Tile program over its 4 heads (2 pairs).

v2 redesign (vs 88us baseline):
  - Triangle masking moved from DVE tensor_muls to PE "penalty matmuls":
    identity-weight x [L|U] (-240) pattern accumulated into the score PSUM.
    exp(score-240*SCALE) ~ 0, so masked cells vanish in both PV numerator
    and the ones-column denominator.  Saves ~17us of DVE time.
  - The two heads of a pair share one score PSUM tile [128,1536] (3 banks,
    2 bufs): head A in cols 0:512 + 1024:1152, head B 512:1024 + 1152:1280.
    exp becomes ONE wide instruction per (pair, kb) instead of two,
    amortizing the ~352cy ACT instruction overhead.
  - exp is split between ACT (true exp, leading columns) and DVE
    (Schraudolph bit-trick: bits = round(score*SCALE*184.665 + 16250.5)
    as int16 == bf16 bits of exp; ~3% per-element, ~4e-3 end-to-end).
  - Normalization merged: one reciprocal [128,4] + one 4D tensor_mul per
    2 query blocks covering both heads.
  - Staging casts moved off DVE (ACT during bootstrap, GPSIMD in feed).
"""

import os
import sys
from contextlib import ExitStack

import numpy as np

sys.path.insert(0, "/opt/trn_rl_repo")

import concourse.bacc as bacc
import concourse.tile as tile
from concourse import mybir
from concourse.bass_utils import run_bass_kernel_spmd

F32 = mybir.dt.float32
BF16 = mybir.dt.bfloat16
I16 = mybir.dt.int16
EXP = mybir.ActivationFunctionType.Exp
MULT = mybir.AluOpType.mult
ADD = mybir.AluOpType.add

B, H, T, D = 2, 16, 2048, 64
WINDOW = 512
SCALE = D ** -0.5
N_CORES = 8
HEADS_PER_CORE = (B * H) // N_CORES  # 4
TB = T // 128  # 16 query/key blocks
TH = 1024  # half-sequence tile width for qd/kd

# Schraudolph exp: bf16 bits of exp(t) ~ round(184.665*t + 16250.5)
A_MULT = SCALE * 128 * 1.4426950408889634
A_ADD = 16250.5
PEN = -240.0  # additive pre-scale penalty: exp(PEN*SCALE) = e^-30 ~ 0

# columns of each merged exp tile handled by DVE (Schraudolph); rest on ACT
DVE_COLS = int(os.environ.get("KOPT_DVE_COLS", "320"))
FEED_CAST = os.environ.get("KOPT_FEED_CAST", "dve")
TRICKLE = int(os.environ.get("KOPT_TRICKLE", "2"))
USE_PEN = os.environ.get("KOPT_PEN", "1") == "1"
WIDE_EXP = os.environ.get("KOPT_WIDE", "1") == "1"
NORM4D = os.environ.get("KOPT_NORM4D", "1") == "1"


def ecol(hl, j):
    """Column offset of the 128-wide E chunk for head-local hl (0/1) and
    j = qb - kb (0..4) in the merged [128, 1280] tile."""
    if j == 0:
        return 512 * hl          # diag
    if j == 4:
        return 512 * hl + 128    # window boundary
    if j < 3:
        return 512 * hl + 256 + 128 * (j - 1)  # mid j=1,2
    return 1024 + 128 * hl       # mid j=3


def build_nc(t=T, heads_per_core=HEADS_PER_CORE):
    nb = t // 128

    nc = bacc.Bacc("TRN2", target_bir_lowering=False)
    q_ext = nc.declare_dram_parameter("q", [heads_per_core, t, D], F32, isOutput=False)
    k_ext = nc.declare_dram_parameter("k", [heads_per_core, t, D], F32, isOutput=False)
    v_ext = nc.declare_dram_parameter("v", [heads_per_core, t, D], F32, isOutput=False)
    id_ext = nc.declare_dram_parameter("ident", [128, 128], F32, isOutput=False)
    pen_ext = nc.declare_dram_parameter("pen", [128, 256], F32, isOutput=False)
    o_ext = nc.declare_dram_parameter("out", [heads_per_core, t, D], F32, isOutput=True)

    assert heads_per_core % 2 == 0
    n_pairs = heads_per_core // 2

    with tile.TileContext(nc) as tc, ExitStack() as ctx:
        const = ctx.enter_context(tc.tile_pool(name="const", bufs=1))
        stage = ctx.enter_context(tc.tile_pool(name="stage", bufs=6))
        vstage = ctx.enter_context(tc.tile_pool(name="vstage", bufs=2))
        qkd = ctx.enter_context(tc.tile_pool(name="qkd", bufs=2))
        vps = ctx.enter_context(tc.tile_pool(name="vps", bufs=4))
        ets = ctx.enter_context(tc.tile_pool(name="ets", bufs=7))
        outs = ctx.enter_context(tc.tile_pool(name="outs", bufs=2))
        rcp = ctx.enter_context(tc.tile_pool(name="rcp", bufs=4))
        # PSUM banks: 1 (trp) + 2*3 (sp) + 1 (shared ob/warmup) = 8
        tr_ps = ctx.enter_context(tc.tile_pool(name="tr_ps", bufs=1, space="PSUM"))
        s_ps = ctx.enter_context(tc.tile_pool(name="s_ps", bufs=2, space="PSUM"))
        ob_ps = ctx.enter_context(tc.tile_pool(name="ob_ps", bufs=1, space="PSUM"))

        # HAM warmup: burn a dense burst of dummy matmuls on a scratch PSUM
        # region while the first DMAs are in flight so the PE clock gate
        # opens (1.2 -> 2.4 GHz) before the real pipeline starts.
        dm_src = const.tile([128, 128], BF16, tag="dm_src")
        nc.vector.memset(dm_src[:], 0.0)
        dm_out = ob_ps.tile([128, 512], F32, tag="ob", name="ob_warm")

        def pe_dummy(n):
            for i in range(n):
                nc.tensor.matmul(
                    dm_out[:, 384:512], dm_src[:], dm_src[:], start=True, stop=True
                )

        # fp32 identity + bf16 copy (for Q/K transposes + penalty matmuls).
        ident_f = const.tile([128, 128], F32, tag="ident_f")
        nc.sync.dma_start(out=ident_f[:], in_=id_ext[:])
        ident_b = const.tile([128, 128], BF16, tag="ident_b")
        nc.vector.tensor_copy(ident_b[:], ident_f[:])

        # penalty pattern [L | U]: L = PEN where col<ch (kills q<k in diag),
        # U = PEN where col>=ch (kills out-of-window in win chunk).
        pen_f = const.tile([128, 256], F32, tag="pen_f")
        nc.scalar.dma_start(out=pen_f[:], in_=pen_ext[:])
        pen_b = const.tile([128, 256], BF16, tag="pen_b")
        nc.vector.tensor_copy(pen_b[:], pen_f[:])
        mask01 = const.tile([128, 256], BF16, tag="mask01")
        nc.vector.tensor_scalar(
            mask01[:], pen_f[:], -1.0 / 240.0, 1.0, MULT, ADD
        )

        # per-pair state
        qd_halves = {}
        kd_halves = {}
        vp = {}

        def alloc_pair(pair):
            qd_halves[pair] = [
                qkd.tile([128, TH], BF16, tag="qd0", name=f"qd0_{pair}"),
                qkd.tile([128, TH], BF16, tag="qd1", name=f"qd1_{pair}"),
            ]
            kd_halves[pair] = [
                qkd.tile([128, TH], BF16, tag="kd0", name=f"kd0_{pair}"),
                qkd.tile([128, TH], BF16, tag="kd1", name=f"kd1_{pair}"),
            ]

        def stage_dma(pair, ext, u, engs):
            # DMA one 512-row chunk of q or k (both heads) into a staging
            # tile; issue the two half-DMAs on separate engine queues.
            rows = slice(u * 512, (u + 1) * 512)
            st_f = stage.tile([128, 512], F32, tag="st_f")
            st3 = st_f[:].rearrange("p (b c) -> p b c", c=128)
            for eng, (hh, doff) in zip(
                engs, ((2 * pair, 0), (2 * pair + 1, 64))
            ):
                eng.dma_start(
                    out=st3[:, :, doff : doff + 64],
                    in_=ext[hh, rows, :].rearrange("(b p) d -> p b d", p=128),
                )
            return st_f

        def stage_compute(st_f, halves, u, cast="gps"):
            # cast -> 4 PE transposes -> drain into the d-major half
            st_b = stage.tile([128, 512], BF16, tag="st_b")
            if cast == "act":
                nc.scalar.activation(
                    st_b[:], st_f[:], mybir.ActivationFunctionType.Copy
                )
            elif cast == "gps":
                nc.gpsimd.tensor_copy(st_b[:], st_f[:])
            else:
                nc.vector.tensor_copy(st_b[:], st_f[:])
            trp = tr_ps.tile([128, 512], BF16, tag="trp")
            for i in range(4):
                nc.tensor.transpose(
                    trp[:, i * 128 : (i + 1) * 128],
                    st_b[:, i * 128 : (i + 1) * 128],
                    ident_b[:],
                )
            dst = halves[u // 2]
            dcol = (u % 2) * 512
            nc.vector.tensor_copy(dst[:, dcol : dcol + 512], trp[:, 0:512])

        def stage_unit(pair, ext, halves, u, engs, cast=FEED_CAST):
            stage_compute(stage_dma(pair, ext, u, engs), halves, u, cast=cast)

        def stage_v(h):
            vst = vstage.tile([128, 1024], F32, tag="vst")
            v3 = vst[:].rearrange("p (b d) -> p b d", d=64)
            nc.gpsimd.dma_start(
                out=v3, in_=v_ext[h].rearrange("(b p) d -> p b d", p=128)
            )
            vt = vps.tile([128, nb, 65], BF16, tag="vp", name=f"vp_{h}")
            nc.vector.tensor_copy(vt[:, :, 0:64], v3)
            nc.gpsimd.memset(vt[:, :, 64:65], 1.0)
            vp[h] = vt

        def stage_feed(pair, engs):
            alloc_pair(pair)
            units = []
            units.append(lambda: stage_unit(pair, q_ext, qd_halves[pair], 0, engs))
            units.append(lambda: stage_unit(pair, k_ext, kd_halves[pair], 0, engs))
            units.append(lambda: stage_v(2 * pair))
            units.append(lambda: stage_v(2 * pair + 1))
            units.append(lambda: stage_unit(pair, q_ext, qd_halves[pair], 1, engs))
            units.append(lambda: stage_unit(pair, k_ext, kd_halves[pair], 1, engs))
            for u in (2, 3):
                units.append(
                    lambda u=u: stage_unit(pair, q_ext, qd_halves[pair], u, engs)
                )
                units.append(
                    lambda u=u: stage_unit(pair, k_ext, kd_halves[pair], u, engs)
                )
            return units

        def attention(pair, feed):
            hA, hB = 2 * pair, 2 * pair + 1
            rows_of = {0: slice(0, 64), 1: slice(64, 128)}
            qdh, kdh = qd_halves[pair], kd_halves[pair]
            et = {}
            oo_t = [None]

            def emit_qk(kb):
                a = kb * 128
                has_win = a + 640 <= t
                mw12 = max(0, min(256, t - a - 128))
                mw3 = max(0, min(128, t - a - 384))
                sp = s_ps.tile([128, 1536], F32, tag="sp", name=f"sp_{pair}_{kb}")
                kd_half = kdh[a // TH]
                kcol = a % TH

                # per-head chunk lists: (ecol, qlo, n).  start=True clears the
                # has_written bits of the WHOLE bank, so only the first
                # matmul touching each bank may use it; later chunks
                # overwrite-where-clear / accumulate-where-set.
                def head_chunks(hl):
                    base = 512 * hl
                    ch = [(hl, base, a, 128)]  # diag (bank first; pen closes)
                    if has_win:
                        ch.append((hl, base + 128, a + 512, 128))
                    # mid j=1,2 (split at qd-half boundary)
                    q0 = a + 128
                    rem = mw12
                    c = base + 256
                    while rem > 0:
                        n = min(rem, TH - (q0 % TH))
                        ch.append((hl, c, q0, n))
                        q0 += n; c += n; rem -= n
                    return ch

                # zip A/B chunks for co-execution (they write different
                # banks); the two mid3 chunks share bank 2 and must NOT
                # co-execute (one PE write port per bank), so mid3A is
                # emitted first (adjacent to same-row diagA => serial) and
                # mid3B dead last.
                ordered = []
                if mw3 > 0:
                    ordered.append((0, 1024, a + 384, mw3))
                for ca, cb in zip(head_chunks(0), head_chunks(1)):
                    ordered.append(ca)
                    ordered.append(cb)
                if mw3 > 0:
                    ordered.append((1, 1152, a + 384, mw3))
                bank_started = set()
                for (hl, c, qlo, n) in ordered:
                    bank = c // 512
                    st_ = bank not in bank_started
                    bank_started.add(bank)
                    # diag/win cells are closed by the penalty matmul;
                    # mids close themselves.
                    sp_ = (c % 512) >= 256 or c >= 1024 or not USE_PEN
                    nc.tensor.matmul(
                        sp[:, c : c + n],
                        kd_half[r_ := rows_of[hl], kcol : kcol + 128],
                        qdh[qlo // TH][r_, qlo % TH : qlo % TH + n],
                        start=st_,
                        stop=sp_,
                    )
                # penalty accumulates onto diag+win (overwrites win cols with
                # the pattern when there is no win chunk -- never read then)
                if USE_PEN:
                    for hl in (0, 1):
                        nc.tensor.matmul(
                            sp[:, 512 * hl : 512 * hl + 256],
                            ident_b[:],
                            pen_b[:],
                            start=False,
                            stop=True,
                        )

                # written column runs of the merged tile
                bnd = 256 if (has_win or USE_PEN) else 128
                runs = []
                for base in (0, 512):
                    runs.append((base, base + bnd))
                    if mw12 > 0:
                        runs.append((base + 256, base + 256 + mw12))
                if mw3 > 0:
                    runs.append((1024, 1024 + mw3))
                    runs.append((1152, 1152 + mw3))
                merged = []
                for lo, hi in runs:
                    if merged and merged[-1][1] == lo:
                        merged[-1] = (merged[-1][0], hi)
                    else:
                        merged.append((lo, hi))

                e = ets.tile([128, 1280], BF16, tag="et", name=f"et_{pair}_{kb}")
                et[kb] = e
                for lo, hi in merged:
                    dcols = DVE_COLS if (hi - lo) >= 1024 else 0
                    split = hi - dcols
                    if not WIDE_EXP:
                        # split ACT part at 512-boundaries
                        c0 = lo
                        while c0 < split:
                            c1 = min(split, (c0 // 512 + 1) * 512)
                            nc.scalar.activation(
                                e[:, c0:c1], sp[:, c0:c1], EXP, scale=SCALE
                            )
                            c0 = c1
                    elif split > lo:
                        nc.scalar.activation(
                            e[:, lo:split], sp[:, lo:split], EXP, scale=SCALE
                        )
                    if dcols:
                        nc.vector.tensor_scalar(
                            e[:, split:hi].bitcast(I16),
                            sp[:, split:hi],
                            A_MULT,
                            A_ADD,
                            MULT,
                            ADD,
                        )
                if not USE_PEN:
                    for base in (0, 512):
                        nc.vector.tensor_mul(
                            e[:, base : base + bnd],
                            e[:, base : base + bnd],
                            mask01[:, 0:bnd],
                        )

            ob_t = {}

            def emit_pv(qb):
                g, j4 = qb // 4, qb % 4
                g2, j2 = qb // 2, qb % 2
                jj = (qb % 4) // 2
                for hl in (0, 1):
                    h = 2 * pair + hl
                    if j2 == 0 and hl == 0:
                        ob_t[0] = ob_ps.tile(
                            [128, 512], F32, tag="ob", name=f"ob_{pair}_{g2}"
                        )
                    hoff = 130 * hl
                    ob = ob_t[0][:, hoff : hoff + 130].rearrange(
                        "p (b c) -> p b c", c=65
                    )
                    kb0 = max(0, qb - 4)
                    for kb in range(kb0, qb + 1):
                        c = ecol(hl, qb - kb)
                        nc.tensor.matmul(
                            ob[:, j2, :],
                            et[kb][:, c : c + 128],
                            vp[h][:, kb, :],
                            start=(kb == kb0),
                            stop=(kb == qb),
                        )
                    # HAM-holding dummies.  Safe window: they sit between
                    # this ob group's first and last PV matmuls in PE order,
                    # so every DVE read of the ob bank (normalize of the
                    # previous group, which gated this group's first matmul
                    # via WAR) is already done, and this group's normalize
                    # only starts after the group's last matmul.
                    if hl == 0:
                        pe_dummy(TRICKLE)
                if qb >= 4:
                    del et[qb - 4]
                if j2 == 1:
                    if jj == 0:
                        oo_t[0] = outs.tile(
                            [128, 512], F32, tag="oo", name=f"oo_{pair}_{g}"
                        )
                    ob4 = ob_t[0][:, 0:260].rearrange(
                        "p (h b c) -> p h b c", h=2, c=65
                    )
                    oo4 = oo_t[0][:].rearrange(
                        "p (h b d) -> p h b d", h=2, d=64
                    )
                    if NORM4D:
                        rc = rcp.tile([128, 4], F32, tag="rc")
                        rc2 = rc[:].rearrange("p (h b) -> p h b", h=2)
                        nc.vector.reciprocal(rc2, ob4[:, :, :, 64])
                        nc.vector.tensor_mul(
                            oo4[:, :, 2 * jj : 2 * jj + 2, :],
                            ob4[:, :, :, 0:64],
                            rc[:]
                            .rearrange("p (h b c) -> p h b c", h=2, c=1)
                            .broadcast_to([128, 2, 2, 64]),
                        )
                    else:
                        for hl in (0, 1):
                            rc = rcp.tile([128, 2], F32, tag="rc")
                            nc.vector.reciprocal(rc[:], ob4[:, hl, :, 64])
                            nc.vector.tensor_mul(
                                oo4[:, hl, 2 * jj : 2 * jj + 2, :],
                                ob4[:, hl, :, 0:64],
                                rc[:]
                                .rearrange("p (b c) -> p b c", c=1)
                                .broadcast_to([128, 2, 64]),
                            )
                if j4 == 3:
                    oo4 = oo_t[0][:].rearrange(
                        "p (h b d) -> p h b d", h=2, d=64
                    )
                    for hl in (0, 1):
                        h = 2 * pair + hl
                        o_dst = o_ext[h, g * 512 : g * 512 + 512, :].rearrange(
                            "(b p) d -> p b d", p=128
                        )
                        if pair == n_pairs - 1 and g == 3:
                            nc.sync.dma_start(
                                out=o_dst[:, 0:2, :], in_=oo4[:, hl, 0:2, :]
                            )
                            nc.scalar.dma_start(
                                out=o_dst[:, 2:4, :], in_=oo4[:, hl, 2:4, :]
                            )
                        else:
                            nc.sync.dma_start(out=o_dst, in_=oo4[:, hl, :, :])

            for kb in range(nb + 1):
                if kb < nb:
                    emit_qk(kb)
                if kb >= 1:
                    emit_pv(kb - 1)
                for fn in feed.get(kb, ()):
                    fn()

        # bootstrap: QK(0) needs q rows 0:1024 (u0q,u1q) and k rows 0:512
        # (u0k) -- spread those three units across the sync+scalar HW-DGE
        # queues; V rides the gpsimd SWDGE queue.
        alloc_pair(0)
        stf_q0 = stage_dma(0, q_ext, 0, (nc.sync, nc.scalar))
        stf_k0 = stage_dma(0, k_ext, 0, (nc.sync, nc.scalar))
        stf_q1 = stage_dma(0, q_ext, 1, (nc.sync, nc.scalar))
        stage_v(0)
        stage_v(1)
        pe_dummy(12)
        stage_compute(stf_q0, qd_halves[0], 0, cast="dve")
        pe_dummy(6)
        stage_compute(stf_k0, kd_halves[0], 0, cast="act")
        pe_dummy(6)
        stage_compute(stf_q1, qd_halves[0], 1, cast="dve")
        pe_dummy(6)
        dm_sink = const.tile([128, 1], F32, tag="dm_sink")
        nc.vector.tensor_copy(dm_sink[:], dm_out[:, 384:385])
        eng0 = (nc.sync, nc.gpsimd)
        feed0 = {
            0: [
                lambda: stage_unit(0, k_ext, kd_halves[0], 1, eng0),
                lambda: stage_unit(0, q_ext, qd_halves[0], 2, eng0),
            ],
            1: [lambda: stage_unit(0, k_ext, kd_halves[0], 2, eng0)],
            2: [lambda: stage_unit(0, q_ext, qd_halves[0], 3, eng0)],
            3: [lambda: stage_unit(0, k_ext, kd_halves[0], 3, eng0)],
        }
        # pair 1 staged during pair 0's attention, starting at kb=6
        units1 = stage_feed(1, eng0)
        feed1_in_0 = {6 + i: [units1[i]] for i in range(len(units1))}
        feed0.update(feed1_in_0)

        attention(0, feed0)
        attention(1, {})

    nc.compile()
    return nc


_NC_CACHE = {}
TRACE = False
TRACE_DIR = None
LAST_RESULT = None


def _get_nc():
    key = (T, HEADS_PER_CORE)
    if key not in _NC_CACHE:
        _NC_CACHE[key] = build_nc()
    return _NC_CACHE[key]


def _pen_pattern():
    ch = np.arange(128)[:, None]
    col = np.arange(128)[None, :]
    L = np.where(col < ch, PEN, 0.0).astype(np.float32)
    U = np.where(col >= ch, PEN, 0.0).astype(np.float32)
    return np.concatenate([L, U], axis=1)


def kernel(q, k, v):
    q = np.ascontiguousarray(np.asarray(q, dtype=np.float32))
    k = np.ascontiguousarray(np.asarray(k, dtype=np.float32))
    v = np.ascontiguousarray(np.asarray(v, dtype=np.float32))
    assert q.shape == (B, H, T, D)

    qf = q.reshape(B * H, T, D)
    kf = k.reshape(B * H, T, D)
    vf = v.reshape(B * H, T, D)
    ident = np.eye(128, dtype=np.float32)
    pen = _pen_pattern()

    in_maps = []
    for c in range(N_CORES):
        s = slice(c * HEADS_PER_CORE, (c + 1) * HEADS_PER_CORE)
        in_maps.append(
            {
                "q": np.ascontiguousarray(qf[s]),
                "k": np.ascontiguousarray(kf[s]),
                "v": np.ascontiguousarray(vf[s]),
                "ident": ident,
                "pen": pen,
            }
        )

    nc = _get_nc()
    global LAST_RESULT
    res = run_bass_kernel_spmd(
        nc, in_maps, list(range(N_CORES)), trace=TRACE, tmpdir=TRACE_DIR
    )
    LAST_RESULT = res
    out = np.concatenate([res.results[c]["out"] for c in range(N_CORES)], axis=0)
    return out.reshape(B, H, T, D).astype(np.float32)


# revision 20
# speedup vs baseline: 1.2031x; 1.0335x over previous
"""Sliding-window causal attention (B=2, H=16, T=2048, D=64, WINDOW=512) on
8 TRN2 NeuronCores.

Sharding: the 32 (b, h) pairs are split 4-per-core (embarrassingly parallel).
Each core runs the same Bass/Tile program over its 4 heads (2 pairs).

v2 redesign (vs 88us baseline):
  - Triangle masking moved from DVE tensor_muls to PE "penalty matmuls":
    identity-weight x [L|U] (-240) pattern accumulated into the score PSUM.
    exp(score-240*SCALE) ~ 0, so masked cells vanish in both PV numerator
    and the ones-column denominator.  Saves ~17us of DVE time.
  - The two heads of a pair share one score PSUM tile [128,1536] (3 banks,
    2 bufs): head A in cols 0:512 + 1024:1152, head B 512:1024 + 1152:1280.
    exp becomes ONE wide instruction per (pair, kb) instead of two,
    amortizing the ~352cy ACT instruction overhead.
  - exp is split between ACT (true exp, leading columns) and DVE
    (Schraudolph bit-trick: bits = round(score*SCALE*184.665 + 16250.5)
    as int16 == bf16 bits of exp; ~3% per-element, ~4e-3 end-to-end).
  - Normalization merged: one reciprocal [128,4] + one 4D tensor_mul per
    2 query blocks covering both heads.
  - Staging casts moved off DVE (ACT during bootstrap, GPSIMD in feed).
"""

import os
import sys
from contextlib import ExitStack

import numpy as np

sys.path.insert(0, "/opt/trn_rl_repo")

import concourse.bacc as bacc
import concourse.tile as tile
from concourse import mybir
from concourse.bass_utils import run_bass_kernel_spmd

F32 = mybir.dt.float32
BF16 = mybir.dt.bfloat16
I16 = mybir.dt.int16
EXP = mybir.ActivationFunctionType.Exp
MULT = mybir.AluOpType.mult
ADD = mybir.AluOpType.add

B, H, T, D = 2, 16, 2048, 64
WINDOW = 512
SCALE = D ** -0.5
N_CORES = 8
HEADS_PER_CORE = (B * H) // N_CORES  # 4
TB = T // 128  # 16 query/key blocks
TH = 1024  # half-sequence tile width for qd/kd

# Schraudolph exp: bf16 bits of exp(t) ~ round(184.665*t + 16250.5)
A_MULT = SCALE * 128 * 1.4426950408889634
A_ADD = 16250.5
PEN = -240.0  # additive pre-scale penalty: exp(PEN*SCALE) = e^-30 ~ 0

# columns of each merged exp tile handled by DVE (Schraudolph); rest on ACT
DVE_COLS = int(os.environ.get("KOPT_DVE_COLS", "320"))
FEED_CAST = os.environ.get("KOPT_FEED_CAST", "dve")
TRICKLE = int(os.environ.get("KOPT_TRICKLE", "2"))
USE_PEN = os.environ.get("KOPT_PEN", "1") == "1"
WIDE_EXP = os.environ.get("KOPT_WIDE", "1") == "1"
NORM4D = os.environ.get("KOPT_NORM4D", "1") == "1"


def ecol(hl, j):
    """Column offset of the 128-wide E chunk for head-local hl (0/1) and
    j = qb - kb (0..4) in the merged [128, 1280] tile."""
    if j == 0:
        return 512 * hl          # diag
    if j == 4:
        return 512 * hl + 128    # window boundary
    if j < 3:
        return 512 * hl + 256 + 128 * (j - 1)  # mid j=1,2
    return 1024 + 128 * hl       # mid j=3


def build_nc(t=T, heads_per_core=HEADS_PER_CORE):
    nb = t // 128

    nc = bacc.Bacc("TRN2", target_bir_lowering=False)
    q_ext = nc.declare_dram_parameter("q", [heads_per_core, t, D], F32, isOutput=False)
    k_ext = nc.declare_dram_parameter("k", [heads_per_core, t, D], F32, isOutput=False)
    v_ext = nc.declare_dram_parameter("v", [heads_per_core, t, D], F32, isOutput=False)
    id_ext = nc.declare_dram_parameter("ident", [128, 128], F32, isOutput=False)
    pen_ext = nc.declare_dram_parameter("pen", [128, 256], F32, isOutput=False)
    o_ext = nc.declare_dram_parameter("out", [heads_per_core, t, D], F32, isOutput=True)

    assert heads_per_core % 2 == 0
    n_pairs = heads_per_core // 2

    with tile.TileContext(nc) as tc, ExitStack() as ctx:
        const = ctx.enter_context(tc.tile_pool(name="const", bufs=1))
        stage = ctx.enter_context(tc.tile_pool(name="stage", bufs=12))
        vstage = ctx.enter_context(tc.tile_pool(name="vstage", bufs=2))
        qkd = ctx.enter_context(tc.tile_pool(name="qkd", bufs=2))
        vps = ctx.enter_context(tc.tile_pool(name="vps", bufs=4))
        ets = ctx.enter_context(tc.tile_pool(name="ets", bufs=7))
        outs = ctx.enter_context(tc.tile_pool(name="outs", bufs=2))
        rcp = ctx.enter_context(tc.tile_pool(name="rcp", bufs=4))
        # PSUM banks: 1 (trp) + 2*3 (sp) + 1 (shared ob/warmup) = 8
        tr_ps = ctx.enter_context(tc.tile_pool(name="tr_ps", bufs=1, space="PSUM"))
        s_ps = ctx.enter_context(tc.tile_pool(name="s_ps", bufs=2, space="PSUM"))
        ob_ps = ctx.enter_context(tc.tile_pool(name="ob_ps", bufs=1, space="PSUM"))

        # HAM warmup: burn a dense burst of dummy matmuls on a scratch PSUM
        # region while the first DMAs are in flight so the PE clock gate
        # opens (1.2 -> 2.4 GHz) before the real pipeline starts.
        dm_src = const.tile([128, 128], BF16, tag="dm_src")
        nc.vector.memset(dm_src[:], 0.0)
        dm_out = ob_ps.tile([128, 512], F32, tag="ob", name="ob_warm")

        def pe_dummy(n):
            for i in range(n):
                nc.tensor.matmul(
                    dm_out[:, 384:512], dm_src[:], dm_src[:], start=True, stop=True
                )

        # fp32 identity + bf16 copy (for Q/K transposes + penalty matmuls).
        ident_f = const.tile([128, 128], F32, tag="ident_f")
        nc.sync.dma_start(out=ident_f[:], in_=id_ext[:])
        ident_b = const.tile([128, 128], BF16, tag="ident_b")
        nc.vector.tensor_copy(ident_b[:], ident_f[:])

        # penalty pattern [L | U]: L = PEN where col<ch (kills q<k in diag),
        # U = PEN where col>=ch (kills out-of-window in win chunk).
        pen_f = const.tile([128, 256], F32, tag="pen_f")
        nc.scalar.dma_start(out=pen_f[:], in_=pen_ext[:])
        pen_b = const.tile([128, 256], BF16, tag="pen_b")
        nc.vector.tensor_copy(pen_b[:], pen_f[:])
        mask01 = const.tile([128, 256], BF16, tag="mask01")
        nc.vector.tensor_scalar(
            mask01[:], pen_f[:], -1.0 / 240.0, 1.0, MULT, ADD
        )

        # per-pair state
        qd_halves = {}
        kd_halves = {}
        vp = {}

        def alloc_pair(pair):
            qd_halves[pair] = [
                qkd.tile([128, TH], BF16, tag="qd0", name=f"qd0_{pair}"),
                qkd.tile([128, TH], BF16, tag="qd1", name=f"qd1_{pair}"),
            ]
            kd_halves[pair] = [
                qkd.tile([128, TH], BF16, tag="kd0", name=f"kd0_{pair}"),
                qkd.tile([128, TH], BF16, tag="kd1", name=f"kd1_{pair}"),
            ]

        def stage_dma(pair, ext, u, engs):
            # DMA one 512-row chunk of q or k (both heads) into a staging
            # tile; issue the two half-DMAs on separate engine queues.
            rows = slice(u * 512, (u + 1) * 512)
            st_f = stage.tile([128, 512], F32, tag="st_f")
            st3 = st_f[:].rearrange("p (b c) -> p b c", c=128)
            for eng, (hh, doff) in zip(
                engs, ((2 * pair, 0), (2 * pair + 1, 64))
            ):
                eng.dma_start(
                    out=st3[:, :, doff : doff + 64],
                    in_=ext[hh, rows, :].rearrange("(b p) d -> p b d", p=128),
                )
            return st_f

        def stage_compute(st_f, halves, u, cast="gps"):
            # cast -> 4 PE transposes -> drain into the d-major half
            st_b = stage.tile([128, 512], BF16, tag="st_b")
            if cast == "act":
                nc.scalar.activation(
                    st_b[:], st_f[:], mybir.ActivationFunctionType.Copy
                )
            elif cast == "gps":
                nc.gpsimd.tensor_copy(st_b[:], st_f[:])
            else:
                nc.vector.tensor_copy(st_b[:], st_f[:])
            trp = tr_ps.tile([128, 512], BF16, tag="trp")
            for i in range(4):
                nc.tensor.transpose(
                    trp[:, i * 128 : (i + 1) * 128],
                    st_b[:, i * 128 : (i + 1) * 128],
                    ident_b[:],
                )
            dst = halves[u // 2]
            dcol = (u % 2) * 512
            nc.vector.tensor_copy(dst[:, dcol : dcol + 512], trp[:, 0:512])

        def stage_unit(pair, ext, halves, u, engs, cast=FEED_CAST):
            stage_compute(stage_dma(pair, ext, u, engs), halves, u, cast=cast)

        def stage_v(h):
            vst = vstage.tile([128, 1024], F32, tag="vst")
            v3 = vst[:].rearrange("p (b d) -> p b d", d=64)
            nc.gpsimd.dma_start(
                out=v3, in_=v_ext[h].rearrange("(b p) d -> p b d", p=128)
            )
            vt = vps.tile([128, nb, 65], BF16, tag="vp", name=f"vp_{h}")
            nc.vector.tensor_copy(vt[:, :, 0:64], v3)
            nc.gpsimd.memset(vt[:, :, 64:65], 1.0)
            vp[h] = vt

        def stage_feed(pair, engs):
            alloc_pair(pair)
            units = []
            units.append(lambda: stage_unit(pair, q_ext, qd_halves[pair], 0, engs))
            units.append(lambda: stage_unit(pair, k_ext, kd_halves[pair], 0, engs))
            units.append(lambda: stage_v(2 * pair))
            units.append(lambda: stage_v(2 * pair + 1))
            units.append(lambda: stage_unit(pair, q_ext, qd_halves[pair], 1, engs))
            units.append(lambda: stage_unit(pair, k_ext, kd_halves[pair], 1, engs))
            for u in (2, 3):
                units.append(
                    lambda u=u: stage_unit(pair, q_ext, qd_halves[pair], u, engs)
                )
                units.append(
                    lambda u=u: stage_unit(pair, k_ext, kd_halves[pair], u, engs)
                )
            return units

        def attention(pair, feed):
            hA, hB = 2 * pair, 2 * pair + 1
            rows_of = {0: slice(0, 64), 1: slice(64, 128)}
            qdh, kdh = qd_halves[pair], kd_halves[pair]
            et = {}
            oo_t = [None]

            def emit_qk(kb):
                a = kb * 128
                has_win = a + 640 <= t
                mw12 = max(0, min(256, t - a - 128))
                mw3 = max(0, min(128, t - a - 384))
                sp = s_ps.tile([128, 1536], F32, tag="sp", name=f"sp_{pair}_{kb}")
                kd_half = kdh[a // TH]
                kcol = a % TH

                # per-head chunk lists: (ecol, qlo, n).  start=True clears the
                # has_written bits of the WHOLE bank, so only the first
                # matmul touching each bank may use it; later chunks
                # overwrite-where-clear / accumulate-where-set.
                def head_chunks(hl):
                    base = 512 * hl
                    ch = [(hl, base, a, 128)]  # diag (bank first; pen closes)
                    if has_win:
                        ch.append((hl, base + 128, a + 512, 128))
                    # mid j=1,2 (split at qd-half boundary)
                    q0 = a + 128
                    rem = mw12
                    c = base + 256
                    while rem > 0:
                        n = min(rem, TH - (q0 % TH))
                        ch.append((hl, c, q0, n))
                        q0 += n; c += n; rem -= n
                    return ch

                # zip A/B chunks for co-execution (they write different
                # banks); the two mid3 chunks share bank 2 and must NOT
                # co-execute (one PE write port per bank), so mid3A is
                # emitted first (adjacent to same-row diagA => serial) and
                # mid3B dead last.
                ordered = []
                if mw3 > 0:
                    ordered.append((0, 1024, a + 384, mw3))
                for ca, cb in zip(head_chunks(0), head_chunks(1)):
                    ordered.append(ca)
                    ordered.append(cb)
                if mw3 > 0:
                    ordered.append((1, 1152, a + 384, mw3))
                bank_started = set()
                for (hl, c, qlo, n) in ordered:
                    bank = c // 512
                    st_ = bank not in bank_started
                    bank_started.add(bank)
                    # diag/win cells are closed by the penalty matmul;
                    # mids close themselves.
                    sp_ = (c % 512) >= 256 or c >= 1024 or not USE_PEN
                    nc.tensor.matmul(
                        sp[:, c : c + n],
                        kd_half[r_ := rows_of[hl], kcol : kcol + 128],
                        qdh[qlo // TH][r_, qlo % TH : qlo % TH + n],
                        start=st_,
                        stop=sp_,
                    )
                # penalty accumulates onto diag+win (overwrites win cols with
                # the pattern when there is no win chunk -- never read then)
                if USE_PEN:
                    for hl in (0, 1):
                        nc.tensor.matmul(
                            sp[:, 512 * hl : 512 * hl + 256],
                            ident_b[:],
                            pen_b[:],
                            start=False,
                            stop=True,
                        )

                # written column runs of the merged tile
                bnd = 256 if (has_win or USE_PEN) else 128
                runs = []
                for base in (0, 512):
                    runs.append((base, base + bnd))
                    if mw12 > 0:
                        runs.append((base + 256, base + 256 + mw12))
                if mw3 > 0:
                    runs.append((1024, 1024 + mw3))
                    runs.append((1152, 1152 + mw3))
                merged = []
                for lo, hi in runs:
                    if merged and merged[-1][1] == lo:
                        merged[-1] = (merged[-1][0], hi)
                    else:
                        merged.append((lo, hi))

                e = ets.tile([128, 1280], BF16, tag="et", name=f"et_{pair}_{kb}")
                et[kb] = e
                for lo, hi in merged:
                    dcols = DVE_COLS if (hi - lo) >= 1024 else 0
                    split = hi - dcols
                    if not WIDE_EXP:
                        # split ACT part at 512-boundaries
                        c0 = lo
                        while c0 < split:
                            c1 = min(split, (c0 // 512 + 1) * 512)
                            nc.scalar.activation(
                                e[:, c0:c1], sp[:, c0:c1], EXP, scale=SCALE
                            )
                            c0 = c1
                    elif split > lo:
                        nc.scalar.activation(
                            e[:, lo:split], sp[:, lo:split], EXP, scale=SCALE
                        )
                    if dcols:
                        nc.vector.tensor_scalar(
                            e[:, split:hi].bitcast(I16),
                            sp[:, split:hi],
                            A_MULT,
                            A_ADD,
                            MULT,
                            ADD,
                        )
                if not USE_PEN:
                    for base in (0, 512):
                        nc.vector.tensor_mul(
                            e[:, base : base + bnd],
                            e[:, base : base + bnd],
                            mask01[:, 0:bnd],
                        )

            ob_t = {}

            def emit_pv(qb):
                g, j4 = qb // 4, qb % 4
                g2, j2 = qb // 2, qb % 2
                jj = (qb % 4) // 2
                for hl in (0, 1):
                    h = 2 * pair + hl
                    if j2 == 0 and hl == 0:
                        ob_t[0] = ob_ps.tile(
                            [128, 512], F32, tag="ob", name=f"ob_{pair}_{g2}"
                        )
                    hoff = 130 * hl
                    ob = ob_t[0][:, hoff : hoff + 130].rearrange(
                        "p (b c) -> p b c", c=65
                    )
                    kb0 = max(0, qb - 4)
                    for kb in range(kb0, qb + 1):
                        c = ecol(hl, qb - kb)
                        nc.tensor.matmul(
                            ob[:, j2, :],
                            et[kb][:, c : c + 128],
                            vp[h][:, kb, :],
                            start=(kb == kb0),
                            stop=(kb == qb),
                        )
                    # HAM-holding dummies.  Safe window: they sit between
                    # this ob group's first and last PV matmuls in PE order,
                    # so every DVE read of the ob bank (normalize of the
                    # previous group, which gated this group's first matmul
                    # via WAR) is already done, and this group's normalize
                    # only starts after the group's last matmul.
                    if hl == 0:
                        pe_dummy(TRICKLE)
                if qb >= 4:
                    del et[qb - 4]
                if j2 == 1:
                    if jj == 0:
                        oo_t[0] = outs.tile(
                            [128, 512], F32, tag="oo", name=f"oo_{pair}_{g}"
                        )
                    ob4 = ob_t[0][:, 0:260].rearrange(
                        "p (h b c) -> p h b c", h=2, c=65
                    )
                    oo4 = oo_t[0][:].rearrange(
                        "p (h b d) -> p h b d", h=2, d=64
                    )
                    if NORM4D:
                        rc = rcp.tile([128, 4], F32, tag="rc")
                        rc2 = rc[:].rearrange("p (h b) -> p h b", h=2)
                        nc.vector.reciprocal(rc2, ob4[:, :, :, 64])
                        nc.vector.tensor_mul(
                            oo4[:, :, 2 * jj : 2 * jj + 2, :],
                            ob4[:, :, :, 0:64],
                            rc[:]
                            .rearrange("p (h b c) -> p h b c", h=2, c=1)
                            .broadcast_to([128, 2, 2, 64]),
                        )
                    else:
                        for hl in (0, 1):
                            rc = rcp.tile([128, 2], F32, tag="rc")
                            nc.vector.reciprocal(rc[:], ob4[:, hl, :, 64])
                            nc.vector.tensor_mul(
                                oo4[:, hl, 2 * jj : 2 * jj + 2, :],
                                ob4[:, hl, :, 0:64],
                                rc[:]
                                .rearrange("p (b c) -> p b c", c=1)
                                .broadcast_to([128, 2, 64]),
                            )
                if j4 == 3:
                    oo4 = oo_t[0][:].rearrange(
                        "p (h b d) -> p h b d", h=2, d=64
                    )
                    for hl in (0, 1):
                        h = 2 * pair + hl
                        o_dst = o_ext[h, g * 512 : g * 512 + 512, :].rearrange(
                            "(b p) d -> p b d", p=128
                        )
                        if pair == n_pairs - 1 and g == 3:
                            nc.sync.dma_start(
                                out=o_dst[:, 0:2, :], in_=oo4[:, hl, 0:2, :]
                            )
                            nc.scalar.dma_start(
                                out=o_dst[:, 2:4, :], in_=oo4[:, hl, 2:4, :]
                            )
                        else:
                            nc.sync.dma_start(out=o_dst, in_=oo4[:, hl, :, :])

            for kb in range(nb + 1):
                if kb < nb:
                    emit_qk(kb)
                if kb >= 1:
                    emit_pv(kb - 1)
                for fn in feed.get(kb, ()):
                    fn()

        # bootstrap: DMA ALL of pair 0's q/k upfront, spread across the five
        # DGE queues (vector/tensor queues are idle until the pipeline spins
        # up); V rides the gpsimd SWDGE queue.  Only the compute (cast +
        # transpose + drain) of later units is deferred into the kb loop.
        alloc_pair(0)
        qp_a = (nc.sync, nc.scalar)
        stfs = {}
        stfs["q0"] = stage_dma(0, q_ext, 0, qp_a)
        stfs["k0"] = stage_dma(0, k_ext, 0, qp_a)
        stfs["q1"] = stage_dma(0, q_ext, 1, qp_a)
        stfs["k1"] = stage_dma(0, k_ext, 1, qp_a)
        stfs["q2"] = stage_dma(0, q_ext, 2, qp_a)
        stfs["k2"] = stage_dma(0, k_ext, 2, (nc.sync, nc.gpsimd))
        stfs["q3"] = stage_dma(0, q_ext, 3, (nc.sync, nc.gpsimd))
        stfs["k3"] = stage_dma(0, k_ext, 3, (nc.scalar, nc.gpsimd))
        stage_v(0)
        stage_v(1)
        pe_dummy(12)
        stage_compute(stfs["q0"], qd_halves[0], 0, cast="dve")
        pe_dummy(6)
        stage_compute(stfs["k0"], kd_halves[0], 0, cast="dve")
        pe_dummy(6)
        stage_compute(stfs["q1"], qd_halves[0], 1, cast="dve")
        pe_dummy(6)
        dm_sink = const.tile([128, 1], F32, tag="dm_sink")
        nc.vector.tensor_copy(dm_sink[:], dm_out[:, 384:385])
        feed0 = {
            0: [
                lambda: stage_compute(stfs["k1"], kd_halves[0], 1, cast="dve"),
                lambda: stage_compute(stfs["q2"], qd_halves[0], 2, cast="dve"),
            ],
            1: [lambda: stage_compute(stfs["k2"], kd_halves[0], 2, cast="dve")],
            2: [lambda: stage_compute(stfs["q3"], qd_halves[0], 3, cast="dve")],
            3: [lambda: stage_compute(stfs["k3"], kd_halves[0], 3, cast="dve")],
        }
        # pair 1 staged during pair 0's attention, starting at kb=6
        units1 = stage_feed(1, (nc.sync, nc.gpsimd))
        feed1_in_0 = {6 + i: [units1[i]] for i in range(len(units1))}
        feed0.update(feed1_in_0)

        attention(0, feed0)
        attention(1, {})

    nc.compile()
    return nc


_NC_CACHE = {}
TRACE = False
TRACE_DIR = None
LAST_RESULT = None


def _get_nc():
    key = (T, HEADS_PER_CORE)
    if key not in _NC_CACHE:
        _NC_CACHE[key] = build_nc()
    return _NC_CACHE[key]


def _pen_pattern():
    ch = np.arange(128)[:, None]
    col = np.arange(128)[None, :]
    L = np.where(col < ch, PEN, 0.0).astype(np.float32)
    U = np.where(col >= ch, PEN, 0.0).astype(np.float32)
    return np.concatenate([L, U], axis=1)


def kernel(q, k, v):
    q = np.ascontiguousarray(np.asarray(q, dtype=np.float32))
    k = np.ascontiguousarray(np.asarray(k, dtype=np.float32))
    v = np.ascontiguousarray(np.asarray(v, dtype=np.float32))
    assert q.shape == (B, H, T, D)

    qf = q.reshape(B * H, T, D)
    kf = k.reshape(B * H, T, D)
    vf = v.reshape(B * H, T, D)
    ident = np.eye(128, dtype=np.float32)
    pen = _pen_pattern()

    in_maps = []
    for c in range(N_CORES):
        s = slice(c * HEADS_PER_CORE, (c + 1) * HEADS_PER_CORE)
        in_maps.append(
            {
                "q": np.ascontiguousarray(qf[s]),
                "k": np.ascontiguousarray(kf[s]),
                "v": np.ascontiguousarray(vf[s]),
                "ident": ident,
                "pen": pen,
            }
        )

    nc = _get_nc()
    global LAST_RESULT
    res = run_bass_kernel_spmd(
        nc, in_maps, list(range(N_CORES)), trace=TRACE, tmpdir=TRACE_DIR
    )
    LAST_RESULT = res
    out = np.concatenate([res.results[c]["out"] for c in range(N_CORES)], axis=0)
    return out.reshape(B, H, T, D).astype(np.float32)
